# revision 60
# baseline (speedup 1.0000x reference)
"""Bipartite GNN message passing on 8 Trainium2 NeuronCores.

Math reformulation: relu(h[idx] @ W + b) == relu(h @ W + b)[idx], so each
direction-layer is: per-node message MLP (z) -> gather z rows by edge ->
segment-sum -> update MLP.  Sharding: aggregation-side nodes are split into
8 contiguous ranges (one per core); each core owns ALL edges targeting its
range, so it computes complete aggregates locally (no AllReduce).  Only the
small per-shard z tensors are AllGathered (fp16) each direction-layer, in
4 (src) / 2 (dst) pieces.  Each sweep runs one STAGE PER TABLE (slot order
(q, w, si)): stages accumulate per-window partials into PSUM and merge them
into one staged SBUF buffer; the last stage adds its partial through the
update MLP (3 accumulating matmuls) and emits the NEXT direction's z for
the window's freshly-updated h columns.  AllGather triggers are placed at
sweep stage positions where (a) their z input already flushed, (b) the CC
core is free (a collective trigger blocks the in-order gpsimd queue until
the PREVIOUS collective completes), and (c) desc-gen backlog from earlier
tables hides the ~50-90us CC mesh latency, so the Q7 desc-gen pairs rarely
drain at sweep transitions.

Host-side degree-aware bin packing: shards padded to 12800/6400 so piece
boundaries align with 128-slot bins and mean SPMD group load (1000) sits
just under the 1024-slot chunk quantum; nodes are snake-dealt by degree to
(core, piece) then greedily vector-packed into bins balancing per-(bin,
other-side-table) counts -> padding +2.4% (vs +12.8% contiguous) with
near-perfect core balance.

Segment-sum on the tensor engine: for each 128-edge chunk a one-hot
S[e,j] = (dst_local[e] == j) matrix (DVE is_equal against an iota tile)
turns the segment sum into psum[64f, 128dst] += z_gathered[128e, 64f].T @
S[128e, 128dst], accumulated in a PSUM bank per (table, 512-window) group.

Gathers use dma_gather (SWDGE): int16 indices into <=25600-row tables (the
AllGathered z pieces), 256B padded fp16 rows.  Desc-gen runs ~7.9ns/idx on
the Q7 pair selected by queue_num, asynchronously after dispatch; 4 queues
round-robin give ~2ns/idx aggregate, the kernel's floor (~2.1ms for 1.024M
padded edge-slots over 5 sweeps).  Small DMAs are batched because the sync
sequencer costs ~600ns per dma_start dispatch, in order.

SPMD: one NEFF for all 8 cores; the chunk schedule is the max over cores;
cores pad with (idx=0, dst_local=sentinel) edges that contribute zero.
"""
import numpy as np

D = 64
CHUNK = 128
SUB = 128
WIN = 512
SENT = 999.0
N_CORES = 8
GCAP = 4608  # max slots per dma_gather
SB = 8       # chunks per S-build batch
NQ = 4       # SWDGE queues


class Cfg:
    def __init__(self, ns_pad, nd_pad):
        self.NS_PAD, self.ND_PAD = ns_pad, nd_pad
        self.SRC_SH, self.DST_SH = ns_pad // N_CORES, nd_pad // N_CORES
        # gather-side piece/table geometry: src z is AllGathered in 4 pieces
        # (piece == table), dst z in 2 pieces (piece == table)
        self.S_QTR = self.SRC_SH // 4            # rows/core per AG piece (src)
        self.D_HALF = self.DST_SH // 2           # rows/core per AG piece (dst)
        self.S_TAB = self.S_QTR * N_CORES        # rows per src piece/table
        self.D_TAB = self.D_HALF * N_CORES       # rows per dst piece/table
        assert self.S_TAB <= 32767 and self.D_TAB <= 32767
        assert self.SRC_SH % CHUNK == 0 and self.DST_SH % CHUNK == 0


REAL_CFG = Cfg(102400, 51200)


def _snake_buckets(deg, n_buckets, cap):
    """Deal nodes (by degree desc) snake-wise into n_buckets of size cap."""
    order = np.argsort(-deg, kind="stable")
    fwd = np.arange(n_buckets)
    pat = np.concatenate([fwd, fwd[::-1]])
    rounds = (len(deg) + n_buckets - 1) // n_buckets
    seq = np.tile(pat, rounds // 2 + 1)[:len(deg)]
    bucket = np.empty(len(deg), np.int64)
    bucket[order] = seq
    return bucket


def _pack_bins(node_q_deg, n_bins, cap=CHUNK):
    """Assign nodes (rows = per-table edge counts) to n_bins bins of cap
    slots, greedily minimizing sum of squared per-(bin, table) loads so every
    SPMD (table, window, sub) group lands just under a multiple of CHUNK."""
    n, nq = node_q_deg.shape
    assert n == n_bins * cap
    deg = node_q_deg.sum(1)
    order = np.argsort(-deg, kind="stable")
    cnt = np.zeros((n_bins, nq), np.int64)
    used = np.zeros(n_bins, np.int64)
    slot = np.empty(n, np.int64)
    for i in order:
        dq = node_q_deg[i]
        if deg[i] == 0:
            b = int(np.argmin(used))
        else:
            score = cnt @ dq + (used >= cap) * (1 << 60)
            b = int(np.argmin(score))
        slot[i] = b * cap + used[b]
        used[b] += 1
        cnt[b] += dq
    return slot


def _node_positions(cfg, src_idx, dst_idx):
    """Degree-aware permutations node -> padded slot for both sides.

    Stage A fixes each node's (core, piece) via degree snake-dealing (piece
    == its z table).  Stage B packs nodes into 128-slot si bins within the
    piece, balancing per-(bin, other-side-table) counts.  Any permutation is
    legal: x / plans / output are permuted consistently on the host."""
    deg_s = np.bincount(src_idx, minlength=cfg.NS_PAD)
    deg_d = np.bincount(dst_idx, minlength=cfg.ND_PAD)
    buck_s = _snake_buckets(deg_s, N_CORES * 4, cfg.S_QTR)
    buck_d = _snake_buckets(deg_d, N_CORES * 2, cfg.D_HALF)
    dqd = np.zeros((cfg.ND_PAD, 4), np.int64)
    np.add.at(dqd, (dst_idx, buck_s[src_idx] % 4), 1)
    dqs = np.zeros((cfg.NS_PAD, 2), np.int64)
    np.add.at(dqs, (src_idx, buck_d[dst_idx] % 2), 1)
    pos_d = np.empty(cfg.ND_PAD, np.int64)
    for b in range(N_CORES * 2):
        c, p = divmod(b, 2)
        nodes = np.flatnonzero(buck_d == b)
        slot = _pack_bins(dqd[nodes], cfg.D_HALF // CHUNK)
        pos_d[nodes] = c * cfg.DST_SH + p * cfg.D_HALF + slot
    pos_s = np.empty(cfg.NS_PAD, np.int64)
    for b in range(N_CORES * 4):
        c, p = divmod(b, 4)
        nodes = np.flatnonzero(buck_s == b)
        slot = _pack_bins(dqs[nodes], cfg.S_QTR // CHUNK)
        pos_s[nodes] = c * cfg.SRC_SH + p * cfg.S_QTR + slot
    return pos_s, pos_d


def _src_table_map(cfg, g):
    """src node g -> (table q in 0..3, row in table).  piece p == q."""
    c = g // cfg.SRC_SH
    off = g % cfg.SRC_SH
    q = off // cfg.S_QTR
    row = c * cfg.S_QTR + (off % cfg.S_QTR)
    return q, row


def _dst_table_map(cfg, g):
    """dst node g -> (table q in 0..1, row in table).  piece p == q."""
    c = g // cfg.DST_SH
    off = g % cfg.DST_SH
    p = off // cfg.D_HALF
    row = c * cfg.D_HALF + (off % cfg.D_HALF)
    return p, row


def _build_plan(cfg, gather_idx, seg_idx, table_map, n_tab, shard):
    """SPMD-uniform edge plan for one direction, STAGED BY TABLE.

    Slot order: (table q, window512, sub128, chunk).  The sweep runs one
    stage per table: stages 0..n_tab-2 accumulate each window's partial
    aggregate into PSUM and stage it to SBUF; the last stage adds its
    partials and runs the update MLP.  Gathers from the last-landing
    AllGather piece (== last table) thus sit at the end of the issue order
    with a whole stage of desc-gen backlog in front of them.

    Returns per-core idx16 [128, TOT/16] int16 and dw [128, TOT/128] fp16,
    plus uniform stages [(q, w, [(si, k)])] and gathers [(q, slot0, nsl)].
    """
    n_w = (shard + WIN - 1) // WIN
    n_si_tot = shard // SUB
    q_all, row_all = table_map(cfg, gather_idx)
    core_of = seg_idx // shard
    per_core = []
    nsi_of_w = [min(WIN // SUB, n_si_tot - w * (WIN // SUB))
                for w in range(n_w)]
    Kmax = np.zeros((n_tab, n_w, WIN // SUB), np.int64)
    for c in range(N_CORES):
        m = core_of == c
        row = row_all[m]
        q = q_all[m]
        s = seg_idx[m] - c * shard
        w = s // WIN
        si = (s % WIN) // SUB
        key = (q * n_w + w) * (WIN // SUB) + si
        order = np.argsort(key, kind="stable")
        row, q, s, w, si = row[order], q[order], s[order], w[order], si[order]
        per_core.append((row, s % SUB, w, q, si))
        cnt = np.zeros((n_tab, n_w, WIN // SUB), np.int64)
        np.add.at(cnt, (q, w, si), 1)
        Kmax = np.maximum(Kmax, (cnt + CHUNK - 1) // CHUNK)

    # every (q, w, si) needs >=1 chunk so its staged PSUM range is written
    for q in range(n_tab):
        for w in range(n_w):
            for si in range(nsi_of_w[w]):
                if Kmax[q, w, si] == 0:
                    Kmax[q, w, si] = 1

    slots_per_group = Kmax * CHUNK
    starts = np.zeros_like(slots_per_group)
    total = 0
    for q in range(n_tab):
        for w in range(n_w):
            for si in range(nsi_of_w[w]):
                starts[q, w, si] = total
                total += int(slots_per_group[q, w, si])
    assert total % CHUNK == 0

    idx16_list, dw_list = [], []
    for c in range(N_CORES):
        row, dwv, w, q, si = per_core[c]
        G = np.zeros(total, np.int32)
        DW = np.full(total, SENT, np.float32)
        kk = (q * n_w + w) * (WIN // SUB) + si
        bounds = np.flatnonzero(np.diff(kk)) + 1
        for grp in np.split(np.arange(len(row)), bounds):
            if len(grp) == 0:
                continue
            qq, ww, sg = int(q[grp[0]]), int(w[grp[0]]), int(si[grp[0]])
            st = int(starts[qq, ww, sg])
            n = len(grp)
            G[st:st + n] = row[grp]
            DW[st:st + n] = dwv[grp]
        i16 = np.empty((128, total // 16), np.int16)
        base = G.astype(np.int16).reshape(total // 16, 16).T
        for k in range(8):
            i16[16 * k:16 * (k + 1)] = base
        dw = DW.astype(np.float16).reshape(total // CHUNK, CHUNK).T
        idx16_list.append(i16)
        dw_list.append(np.ascontiguousarray(dw))

    stages = []
    for q in range(n_tab):
        for w in range(n_w):
            groups = [(si, int(Kmax[q, w, si]))
                      for si in range(nsi_of_w[w])]
            stages.append((q, w, groups))

    gathers = []
    for q in range(n_tab):
        for w in range(n_w):
            lo = int(starts[q, w, 0])
            hi = lo + int(slots_per_group[q, w, :nsi_of_w[w]].sum())
            p = lo
            while p < hi:
                n = min(GCAP, hi - p)
                gathers.append((q, w, p, n))
                p += n
    return dict(idx16=idx16_list, dw=dw_list, stages=stages, gathers=gathers,
                total=total, n_w=n_w, n_tab=n_tab)


def _host_prep(cfg, inputs):
    f32 = np.float32
    x_src = np.asarray(inputs["x_src"], f32)
    x_dst = np.asarray(inputs["x_dst"], f32)
    src_idx = np.asarray(inputs["src_idx"]).astype(np.int64)
    dst_idx = np.asarray(inputs["dst_idx"]).astype(np.int64)
    L = np.asarray(inputs["W_msg_sd"]).shape[0]

    ns, nd = x_src.shape[0], x_dst.shape[0]
    pos_s, pos_d = _node_positions(cfg, src_idx, dst_idx)
    xs = np.zeros((cfg.NS_PAD, D), f32)
    xs[pos_s[:ns]] = x_src
    xd = np.zeros((cfg.ND_PAD, D), f32)
    xd[pos_d[:nd]] = x_dst

    plan_sd = _build_plan(cfg, pos_s[src_idx], pos_d[dst_idx],
                          _src_table_map, 4, cfg.DST_SH)
    plan_ds = _build_plan(cfg, pos_d[dst_idx], pos_s[src_idx],
                          _dst_table_map, 2, cfg.SRC_SH)

    def stack_wb(wk, bk):
        w = np.asarray(inputs[wk], f32)
        b = np.asarray(inputs[bk], f32)
        out = np.empty((L, D + 1, D), np.float16)
        out[:, :D] = w.astype(np.float16)
        out[:, D] = b.astype(np.float16)
        return out

    host = dict(
        L=L,
        Wbm_sd=stack_wb("W_msg_sd", "b_msg_sd"),
        Wbm_ds=stack_wb("W_msg_ds", "b_msg_ds"),
        Wu_dst=np.asarray(inputs["W_upd_dst"], f32).astype(np.float16),
        Wu_src=np.asarray(inputs["W_upd_src"], f32).astype(np.float16),
        bu_dst=np.asarray(inputs["b_upd_dst"], f32)[:, :, None],
        bu_src=np.asarray(inputs["b_upd_src"], f32)[:, :, None],
        Win_src=np.asarray(inputs["W_in_src"], f32),
        Win_dst=np.asarray(inputs["W_in_dst"], f32),
        bin_src=np.asarray(inputs["b_in_src"], f32)[:, None],
        bin_dst=np.asarray(inputs["b_in_dst"], f32)[:, None],
        iota=np.tile(np.arange(WIN, dtype=np.float16), (128, 1)),
        xsT=[np.ascontiguousarray(xs[c * cfg.SRC_SH:(c + 1) * cfg.SRC_SH].T)
             for c in range(N_CORES)],
        xdT=[np.ascontiguousarray(xd[c * cfg.DST_SH:(c + 1) * cfg.DST_SH].T)
             for c in range(N_CORES)],
        plan_sd=plan_sd, plan_ds=plan_ds, pos_d=pos_d,
    )
    return host


def _build_nc(cfg, host):
    import concourse.bass as bass
    import concourse.tile as tile
    from concourse import bacc, mybir

    dt = mybir.dt
    L = host["L"]
    plan_sd, plan_ds = host["plan_sd"], host["plan_ds"]
    TOT_SD, TOT_DS = plan_sd["total"], plan_ds["total"]

    nc = bacc.Bacc("TRN2", target_bir_lowering=False, debug=False,
                   num_devices=N_CORES, num_swdge_queues=NQ)

    def inp(name, shape, dtype):
        return nc.dram_tensor(name, shape, dtype, kind="ExternalInput").ap()

    xT_src = inp("xT_src", [D, cfg.SRC_SH], dt.float32)
    xT_dst = inp("xT_dst", [D, cfg.DST_SH], dt.float32)
    Win_src = inp("Win_src", [D, D], dt.float32)
    Win_dst = inp("Win_dst", [D, D], dt.float32)
    bin_src = inp("bin_src", [D, 1], dt.float32)
    bin_dst = inp("bin_dst", [D, 1], dt.float32)
    Wbm_sd = inp("Wbm_sd", [L, D + 1, D], dt.float16)
    Wbm_ds = inp("Wbm_ds", [L, D + 1, D], dt.float16)
    Wu_dst = inp("Wu_dst", [L, 2 * D, D], dt.float16)
    Wu_src = inp("Wu_src", [L, 2 * D, D], dt.float16)
    bu_dst = inp("bu_dst", [L, D, 1], dt.float32)
    bu_src = inp("bu_src", [L, D, 1], dt.float32)
    iota_in = inp("iota", [128, WIN], dt.float16)
    idx_sd = inp("idx_sd", [128, TOT_SD // 16], dt.int16)
    dw_sd = inp("dw_sd", [128, TOT_SD // CHUNK], dt.float16)
    idx_ds = inp("idx_ds", [128, TOT_DS // 16], dt.int16)
    dw_ds = inp("dw_ds", [128, TOT_DS // CHUNK], dt.float16)
    out_hd = nc.dram_tensor("out_hd", [D, cfg.DST_SH], dt.float16,
                            kind="ExternalOutput").ap()

    # per-piece z shard (local) and AllGathered piece tensors (shared)
    zs_sh = [nc.dram_tensor(f"zs_sh{p}", [cfg.S_QTR, 128], dt.float16).ap()
             for p in range(4)]
    zd_sh = [nc.dram_tensor(f"zd_sh{p}", [cfg.D_HALF, 128], dt.float16).ap()
             for p in range(2)]
    zs_pc = [nc.dram_tensor(f"zs_pc{p}", [cfg.S_TAB, 128], dt.float16,
                            addr_space="Shared").ap() for p in range(4)]
    zd_pc = [nc.dram_tensor(f"zd_pc{p}", [cfg.D_TAB, 128], dt.float16,
                            addr_space="Shared").ap() for p in range(2)]

    RELU = mybir.ActivationFunctionType.Relu
    EQ = mybir.AluOpType.is_equal
    rg = [list(range(N_CORES))]

    with tile.TileContext(nc) as tc:
        from contextlib import ExitStack
        with ExitStack() as ctx:
            pers = ctx.enter_context(tc.tile_pool(name="pers", bufs=1))
            ps_agg = ctx.enter_context(
                tc.tile_pool(name="psagg", bufs=4, space="PSUM"))
            ps_mlp = ctx.enter_context(
                tc.tile_pool(name="psmlp", bufs=3, space="PSUM"))
            gath = ctx.enter_context(tc.tile_pool(name="gath", bufs=11))
            idxg = ctx.enter_context(tc.tile_pool(name="idxg", bufs=10))
            spool = ctx.enter_context(tc.tile_pool(name="spool", bufs=4))
            work = ctx.enter_context(tc.tile_pool(name="work", bufs=4))

            h_s = pers.tile([D + 1, cfg.SRC_SH], dt.float16, name="h_s")
            h_d = pers.tile([D + 1, cfg.DST_SH], dt.float16, name="h_d")
            iota_t = pers.tile([128, WIN], dt.float16)
            dw_sd_t = pers.tile([128, TOT_SD // CHUNK], dt.float16)
            dw_ds_t = pers.tile([128, TOT_DS // CHUNK], dt.float16)

            # --- critical-path startup loads ONLY (the sync sequencer costs
            # ~600ns per dma_start dispatch, in order, so everything the
            # encoder-src -> z0 -> AG chain does not need is emitted later)
            w_enc_s = pers.tile([D, D], dt.float32)
            w_enc_d = pers.tile([D, D], dt.float32)
            b_enc_s = pers.tile([D, 1], dt.float32)
            b_enc_d = pers.tile([D, 1], dt.float32)
            nc.sync.dma_start(out=w_enc_s[:], in_=Win_src[:])
            nc.sync.dma_start(out=b_enc_s[:], in_=bin_src[:])
            wbm_t, wu_t, bu_t = {}, {}, {}
            t = pers.tile([D + 1, D], dt.float16, name="wbm_sd0")
            nc.sync.dma_start(out=t[:], in_=Wbm_sd[0])
            wbm_t["sd", 0] = t
            nc.vector.memset(h_s[D:D + 1, :], 1.0)

            # one-time zero fill of z-shard pad columns, one batched DMA per
            # shard tensor (never written later)
            zeros_t = pers.tile([128, 25 * D], dt.float16, name="zeros_t")
            nc.vector.memset(zeros_t[:], 0.0)
            for z_list in (zs_sh, zd_sh):
                for z in z_list:
                    nc.sync.dma_start(
                        out=z[:, D:128].rearrange("(b p) f -> p b f", p=CHUNK),
                        in_=zeros_t[:].rearrange("p (b f) -> p b f", f=D))

            def load_rest():
                nc.sync.dma_start(out=iota_t[:], in_=iota_in[:])
                nc.sync.dma_start(out=dw_sd_t[:], in_=dw_sd[:])
                nc.sync.dma_start(out=dw_ds_t[:], in_=dw_ds[:])
                nc.sync.dma_start(out=w_enc_d[:], in_=Win_dst[:])
                nc.sync.dma_start(out=b_enc_d[:], in_=bin_dst[:])
                for l in range(L):
                    for key, src in (("sd", Wbm_sd), ("ds", Wbm_ds)):
                        if (key, l) in wbm_t:
                            continue
                        t = pers.tile([D + 1, D], dt.float16,
                                      name=f"wbm_{key}{l}")
                        nc.sync.dma_start(out=t[:], in_=src[l])
                        wbm_t[key, l] = t
                    for key, src in (("dst", Wu_dst), ("src", Wu_src)):
                        th = pers.tile([D, D], dt.float16, name=f"wuh_{key}{l}")
                        ta = pers.tile([D, D], dt.float16, name=f"wua_{key}{l}")
                        nc.sync.dma_start(out=th[:], in_=src[l, 0:D, :])
                        nc.sync.dma_start(out=ta[:], in_=src[l, D:2 * D, :])
                        wu_t[key, l] = (th, ta)
                    for key, src in (("dst", bu_dst), ("src", bu_src)):
                        t = pers.tile([D, 1], dt.float32, name=f"bu_{key}{l}")
                        nc.sync.dma_start(out=t[:], in_=src[l])
                        bu_t[key, l] = t
                nc.vector.memset(h_d[D:D + 1, :], 1.0)

            def encoder(xT, w_t, b_t, h_out, n, hooks=None):
                for j0 in range(0, n, WIN):
                    w = min(WIN, n - j0)
                    xs = work.tile([D, WIN], dt.float32, tag="xs")
                    nc.sync.dma_start(out=xs[:, :w], in_=xT[:, j0:j0 + w])
                    ps = ps_mlp.tile([D, WIN], dt.float32, tag="mlp")
                    nc.tensor.matmul(out=ps[:, :w], lhsT=w_t[:], rhs=xs[:, :w],
                                     start=True, stop=True)
                    nc.scalar.activation(out=h_out[0:D, j0:j0 + w],
                                         in_=ps[:, :w], func=RELU, bias=b_t[:])
                    if hooks and j0 // WIN in hooks:
                        hooks[j0 // WIN]()

            def z_piece(h_in, wbm, z_sh, half, p):
                # 5 chunks per PSUM/relu/DMA batch: one dma_start per 640
                # rows (sync-sequencer dispatch is ~600ns per dma_start)
                r0 = p * half
                ZB = 5
                assert half % (ZB * CHUNK) == 0
                for k0 in range(0, half, ZB * CHUNK):
                    ps = ps_mlp.tile([CHUNK, ZB * D], dt.float32, tag="mlp")
                    for b in range(ZB):
                        k = r0 + k0 + b * CHUNK
                        nc.tensor.matmul(
                            out=ps[:, b * D:(b + 1) * D],
                            lhsT=h_in[0:D + 1, k:k + CHUNK],
                            rhs=wbm[:], start=True, stop=True,
                            skip_group_check=True)
                    zs = work.tile([CHUNK, ZB * D], dt.float16, tag="zstage")
                    nc.vector.tensor_scalar_max(out=zs[:], in0=ps[:],
                                                scalar1=0.0)
                    nc.sync.dma_start(
                        out=z_sh[k0:k0 + ZB * CHUNK, 0:D].rearrange(
                            "(b p) f -> p b f", p=CHUNK),
                        in_=zs[:].rearrange("p (b f) -> p b f", f=D))

            def z_win(h_in, wbm, z_shards, piece_rows, w0, ww):
                # z for one window's freshly-updated h columns, emitted right
                # after the window's activation so each AG piece's z finishes
                # with its LAST window's update, not at the sweep's PE tail
                nchunk = ww // CHUNK
                ps = ps_mlp.tile([CHUNK, 4 * D], dt.float32, tag="mlp")
                for b in range(nchunk):
                    k = w0 + b * CHUNK
                    nc.tensor.matmul(
                        out=ps[:, b * D:(b + 1) * D],
                        lhsT=h_in[0:D + 1, k:k + CHUNK],
                        rhs=wbm[:], start=True, stop=True,
                        skip_group_check=True)
                zs = work.tile([CHUNK, 4 * D], dt.float16, tag="zstage")
                nc.vector.tensor_scalar_max(out=zs[:, :nchunk * D],
                                            in0=ps[:, :nchunk * D],
                                            scalar1=0.0)
                b = 0
                while b < nchunk:
                    k = w0 + b * CHUNK
                    p = k // piece_rows
                    lo = k % piece_rows
                    nb = min(nchunk - b, (piece_rows - lo) // CHUNK)
                    nc.sync.dma_start(
                        out=z_shards[p][lo:lo + nb * CHUNK, 0:D].rearrange(
                            "(b p) f -> p b f", p=CHUNK),
                        in_=zs[:].rearrange(
                            "p (b f) -> p b f", f=D)[:, b:b + nb, :])
                    b += nb


            def ag(z_sh, z_pc, p):
                nc.gpsimd.collective_compute(
                    "AllGather", mybir.AluOpType.bypass, replica_groups=rg,
                    ins=[z_sh[p].opt()], outs=[z_pc[p].opt()])

            # staged per-window partial aggregates for all but the last
            # stage, fp16 in SBUF; stages up to n_tab-2 accumulate into
            # slice 0 (DVE add), stage n_tab-2 into slice 1, so both
            # directions need only 2 slices: sd 2*DST_SH, ds 1*SRC_SH cols
            agg_st = pers.tile([D, 2 * cfg.DST_SH], dt.float16, name="agg_st")

            def sweep(plan, tab_of, idx_dram, dw_t, h_io, wu, bu,
                      shard, last=False, hooks=None, preludes=None,
                      z_emit=None):
                # One stage per table: stages accumulate each window's partial
                # aggregate in PSUM and stage it to agg_st; the LAST stage adds
                # its partials via the update MLP (1 + n_tab accumulating
                # matmuls) and writes h.  Gathers are emitted in slot order
                # (q, w, si), incrementally with the stage loop; preludes
                # (AllGather dispatches for THIS sweep's tables) are emitted at
                # early stage positions so they sit mid-stream in the in-order
                # gpsimd queue with desc-gen backlog behind them.
                gathers = plan["gathers"]
                n_g = len(gathers)
                n_w, n_tab = plan["n_w"], plan["n_tab"]
                preludes = preludes or {}
                need_pos = [0] * (n_tab * n_w)
                for g, (qg, wg, s0, nsl) in enumerate(gathers):
                    i = qg * n_w + wg
                    need_pos[i] = max(need_pos[i], g + 1)
                for i in range(1, n_tab * n_w):
                    need_pos[i] = max(need_pos[i], need_pos[i - 1])

                gtiles = [None] * n_g
                cursor = [0]

                def pump(upto):
                    while cursor[0] < min(upto, n_g):
                        g = cursor[0]
                        q, _, s0, nsl = gathers[g]
                        it = idxg.tile([128, nsl // 16], dt.int16, tag="idxg")
                        nc.sync.dma_start(
                            out=it[:],
                            in_=idx_dram[:, s0 // 16:(s0 + nsl) // 16])
                        gt = gath.tile([128, nsl], dt.float16, tag="gt")
                        nc.gpsimd.dma_gather(
                            gt[:].rearrange("p (b e) -> p b e", e=128),
                            tab_of(q), it[:], nsl, nsl, 128,
                            single_packet=False,
                            queue_num=g % NQ)
                        gtiles[g] = (s0, nsl, gt)
                        cursor[0] += 1

                gi = 0
                c = 0  # global chunk cursor
                s_tile = None
                for (q, w, groups) in plan["stages"]:
                    st = q * n_w + w
                    if st in preludes:
                        for fn in preludes[st]:
                            fn()
                    pump(need_pos[min(st + 2, n_tab * n_w - 1)])
                    nvis = sum(k for (_, k) in groups)
                    w0 = w * WIN
                    ww = min(WIN, shard - w0)
                    ps = ps_agg.tile([D, WIN], dt.float32, tag="agg")
                    done = 0
                    for (si, kk) in groups:
                        for _ in range(kk):
                            if c % SB == 0:
                                nb = min(SB, dw_t.shape[1] - c)
                                s_tile = spool.tile([128, nb * SUB],
                                                    dt.float16, tag="s")
                                nc.vector.tensor_tensor(
                                    out=s_tile[:].rearrange(
                                        "p (b e) -> p b e", e=SUB),
                                    in0=dw_t[:, c:c + nb, None].to_broadcast(
                                        [128, nb, SUB]),
                                    in1=iota_t[:, 0:SUB][:, None, :]
                                    .to_broadcast([128, nb, SUB]),
                                    op=EQ)
                            s0, nsl, gt = gtiles[gi]
                            if c * CHUNK >= s0 + nsl:
                                gi += 1
                                s0, nsl, gt = gtiles[gi]
                            blk = (c * CHUNK - s0) // CHUNK
                            g3 = gt[:].rearrange("p (b e) -> p b e", e=128)
                            nc.tensor.matmul(
                                out=ps[:, si * SUB:(si + 1) * SUB],
                                lhsT=g3[:, blk, 0:D],
                                rhs=s_tile[:].rearrange(
                                    "p (b e) -> p b e", e=SUB)[:, c % SB, :],
                                start=(done == 0), stop=(done == nvis - 1),
                                skip_group_check=True)
                            done += 1
                            c += 1
                    if q < n_tab - 1:
                        # stage this table's partial aggregate to SBUF:
                        # stages up to n_tab-2 accumulate into slice 0,
                        # stage n_tab-2 goes to slice 1 (sd only)
                        s_idx = 0 if (n_tab == 2 or q < n_tab - 2) else 1
                        a0 = s_idx * shard + w0
                        if q == 0 or s_idx == 1:
                            nc.vector.tensor_copy(out=agg_st[:, a0:a0 + ww],
                                                  in_=ps[:, :ww])
                        else:
                            nc.vector.tensor_tensor(
                                out=agg_st[:, a0:a0 + ww],
                                in0=agg_st[:, a0:a0 + ww], in1=ps[:, :ww],
                                op=mybir.AluOpType.add)
                    else:
                        agt = work.tile([D, WIN], dt.float16, tag="aggstage")
                        nc.vector.tensor_copy(out=agt[:, :ww], in_=ps[:, :ww])
                        # update MLP: relu(Wh.h + Wa.sum_q agg_q + b)
                        psu = ps_mlp.tile([D, WIN], dt.float32, tag="mlp")
                        nc.tensor.matmul(out=psu[:, :ww], lhsT=wu[0][:],
                                         rhs=h_io[0:D, w0:w0 + ww],
                                         start=True, stop=False,
                                         skip_group_check=True)
                        for s_idx in range(2 if n_tab > 2 else 1):
                            aq = s_idx * shard + w0
                            nc.tensor.matmul(out=psu[:, :ww], lhsT=wu[1][:],
                                             rhs=agg_st[:, aq:aq + ww],
                                             start=False, stop=False,
                                             skip_group_check=True)
                        nc.tensor.matmul(out=psu[:, :ww], lhsT=wu[1][:],
                                         rhs=agt[:, :ww],
                                         start=False, stop=True,
                                         skip_group_check=True)
                        nc.scalar.activation(out=h_io[0:D, w0:w0 + ww],
                                             in_=psu[:, :ww], func=RELU,
                                             bias=bu[:])
                        if last:
                            nc.sync.dma_start(out=out_hd[:, w0:w0 + ww],
                                              in_=h_io[0:D, w0:w0 + ww])
                        if z_emit is not None:
                            z_emit(w0, ww)
                        if hooks and w in hooks:
                            for fn in hooks[w]:
                                fn()
                pump(n_g)

            def sd_tab(q):
                return zs_pc[q]

            def ds_tab(q):
                return zd_pc[q]

            encoder(xT_src, w_enc_s, b_enc_s, h_s, cfg.SRC_SH)
            for p in range(4):
                z_piece(h_s, wbm_t["sd", 0], zs_sh[p], cfg.S_QTR, p)
            load_rest()
            encoder(xT_dst, w_enc_d, b_enc_d, h_d, cfg.DST_SH)

            def hook_w(piece_rows, p):
                return ((p + 1) * piece_rows - 1) // WIN

            n_w_sd = (cfg.DST_SH + WIN - 1) // WIN
            n_w_ds = (cfg.SRC_SH + WIN - 1) // WIN

            # AllGather dispatch schedule.  CC collectives serialize and each
            # has ~50-70us handshake latency, so every AG is emitted at a
            # stage position where (a) its z input has already flushed
            # (emitted >=2 stages earlier), (b) the CC core is free, and
            # (c) desc-gen backlog hides its latency:
            #   zs p0-p2: mid last stage of the producing ds sweep
            #   zs p3:    stage 0 of the consuming sd sweep (hidden by its
            #             own q0-q2 backlog)
            #   zd p0:    near the end of the producing sd sweep's last stage
            #   zd p1:    stage 0 of the consuming ds sweep
            def agz(p):
                return lambda: ag(zs_sh, zs_pc, p)

            def agd(p):
                return lambda: ag(zd_sh, zd_pc, p)

            for l in range(L):
                last = (l == L - 1)
                if l == 0:
                    # startup pieces: z already computed; triggers spaced
                    # ~8 gathers apart because a collective trigger blocks
                    # the gpsimd queue until the PREVIOUS collective completes
                    sd_pre = {0: [agz(0)], 6: [agz(1)],
                              14: [agz(2)], 22: [agz(3)]}
                else:
                    # pieces 2 and 3 trigger here (not at the producing ds
                    # sweep's tail, where the previous collective would still
                    # be running and the blocked trigger would stall the next
                    # sweep's gather stream behind it)
                    sd_pre = {2: [agz(2)], 10: [agz(3)]}
                z_emit = None
                if not last:
                    # this layer's ds-direction z, per window as h_d
                    # finalizes in the sd sweep's last stage
                    z_emit = (lambda w0, ww, l=l:
                              z_win(h_d, wbm_t["ds", l], zd_sh,
                                    cfg.D_HALF, w0, ww))
                    sd_pre.setdefault(3 * n_w_sd + 10, []).append(agd(0))
                sweep(plan_sd, sd_tab, idx_sd, dw_sd_t, h_d,
                      wu_t["dst", l], bu_t["dst", l], cfg.DST_SH,
                      last=last, preludes=sd_pre, z_emit=z_emit)
                if last:
                    break
                # next layer's sd-direction z, per window in the ds sweep's
                # last stage
                z_emit = (lambda w0, ww, l=l:
                          z_win(h_s, wbm_t["sd", l + 1], zs_sh,
                                cfg.S_QTR, w0, ww))
                ds_pre = {8: [agd(1)],
                          n_w_ds + 10: [agz(0)],
                          n_w_ds + 18: [agz(1)]}
                sweep(plan_ds, ds_tab, idx_ds, dw_ds_t, h_s,
                      wu_t["src", l], bu_t["src", l], cfg.SRC_SH,
                      preludes=ds_pre, z_emit=z_emit)

    nc.compile()
    return nc


def make_in_maps(cfg, host):
    shared = dict(
        Win_src=host["Win_src"], Win_dst=host["Win_dst"],
        bin_src=host["bin_src"], bin_dst=host["bin_dst"],
        Wbm_sd=host["Wbm_sd"], Wbm_ds=host["Wbm_ds"],
        Wu_dst=host["Wu_dst"], Wu_src=host["Wu_src"],
        bu_dst=host["bu_dst"], bu_src=host["bu_src"],
        iota=host["iota"],
    )
    maps = []
    for c in range(N_CORES):
        m = dict(shared)
        m["xT_src"] = host["xsT"][c]
        m["xT_dst"] = host["xdT"][c]
        m["idx_sd"] = host["plan_sd"]["idx16"][c]
        m["dw_sd"] = host["plan_sd"]["dw"][c]
        m["idx_ds"] = host["plan_ds"]["idx16"][c]
        m["dw_ds"] = host["plan_ds"]["dw"][c]
        maps.append(m)
    return maps


def kernel(**inputs) -> np.ndarray:
    cfg = REAL_CFG
    host = _host_prep(cfg, inputs)
    nc = _build_nc(cfg, host)
    from concourse.bass_utils import run_bass_kernel_spmd
    res = run_bass_kernel_spmd(nc, make_in_maps(cfg, host),
                               core_ids=list(range(N_CORES)))
    nd = np.asarray(inputs["x_dst"]).shape[0]
    full = np.concatenate([res.results[c]["out_hd"].T
                           for c in range(N_CORES)], axis=0)
    out = full[host["pos_d"][:nd]]
    return out.astype(np.float32)

